# revision 3
# baseline (speedup 1.0000x reference)
"""BiLSTM + prototype-distance kernel for 8 trn2 NeuronCores.

Sharding: 8 cores = 4 batch-shards (8 rows each) x 2 directions.
Backward-direction cores receive time-reversed input ids and run the
identical SPMD program. Host combines per-core partial outputs:
    out = 2*(xp_f + xp_b) - x2_f - x2_b - ||protos||^2.

v2 design notes (vs v0 baseline):
- Gate matmuls issue k-outer / col-group-inner so the four 32-wide PE
  column strips stream concurrently (baseline serialized all 20 MMs).
- xg injection is ONE M=128 selector matmul (start=True covers every
  partition of the bank), so all per-strip gate MMs are pure
  accumulates - no per-group has_written clears.
- phase1 injects the bias with an identity matmul (start=True, M=128),
  the 16 x@Wih MMs accumulate, then one DVE copy lands the granule in
  xg_ring (kills the per-granule DVE bias pass).
- Cell state update: u on DVE, v=f*c on GPSIMD (parallel engines),
  c=2u+v on DVE, tanh on ACT, h=o*tanh(c) on DVE (bf16 out).
- h transpose moved off the PE: DVE stream-transpose (32x32 blocks)
  produces hT2 directly in SBUF; Whh/protos are host-permuted to match
  the block layout (contraction chunk k reads hT2[:, 32k:32k+32],
  partition p <-> hidden 128*(p//32) + 32k + (p%32)).
- ||h||^2 comes free from the PE: 4 extra N=32 Gram matmuls share the
  proto PSUM tile; host reads the diagonal. No DVE square/accum pass.
- proto/Gram MMs for step t issue inside body t+1 (state ping-pong
  buffer), so they fill PE idle time during the activation chain.
"""

import sys
import numpy as np

sys.path.insert(0, "/opt/trn_rl_repo")

import concourse.bass as bass  # noqa: E402
import concourse.tile as tile  # noqa: E402
import concourse.mybir as mybir  # noqa: E402
from concourse import bacc  # noqa: E402
from concourse.bass_utils import run_bass_kernel_spmd  # noqa: E402

F32 = mybir.dt.float32
BF16 = mybir.dt.bfloat16
I32 = mybir.dt.int32

V, E, HD, P = 50000, 512, 1024, 128
H2 = HD // 2          # 512 per-direction hidden
B, T = 32, 512
BS = 8                # batch rows per core
NG = T // 4           # granules (4 timesteps each)
# gate order in G columns: f, i, g, o  (pytorch rows are i,f,g,o)
SRC = [1, 0, 2, 3]
OUTW = 160            # 128 proto cols + 32 gram cols per step


def _arrange_whh(w):
    """w: (2048, 512) -> (4, 128, 2048) bf16-ready tiles in the hT2
    block-transpose convention:
      arr[k][p, 512c + 128gam + j] = w[512*SRC[gam] + 128c + j,
                                       128*(p//32) + 32k + (p%32)]
    (g-gate rows doubled for the 2*sigmoid(2x)-1 tanh trick)."""
    arr = np.empty((4, 128, 2048), np.float32)
    hi = np.arange(128)
    cc, jp = hi // 32, hi % 32
    for k in range(4):
        hin = 128 * cc + 32 * k + jp            # (128,) hidden-in index
        for gam in range(4):
            scale = 2.0 if gam == 2 else 1.0
            # rows 512*SRC[gam] + [0..512) of w, cols hin
            blk = w[512 * SRC[gam]:512 * (SRC[gam] + 1), :][:, hin]  # (512,128)
            for c in range(4):
                arr[k, :, 512 * c + 128 * gam:512 * c + 128 * (gam + 1)] = \
                    scale * blk[128 * c:128 * (c + 1), :].T
    return arr


def _arrange_wih(w):
    """w: (2048, 512) -> (4, 128, 2048): arr[k][kk, 512c+128gam+j] =
    w[512*SRC[gam] + 128c + j, 128k + kk] (*2 on the tanh gate)."""
    arr = np.empty((4, 128, 2048), np.float32)
    for k in range(4):
        for gam in range(4):
            scale = 2.0 if gam == 2 else 1.0
            blk = w[512 * SRC[gam]:512 * (SRC[gam] + 1),
                    128 * k:128 * (k + 1)]      # (512, 128)
            for c in range(4):
                arr[k, :, 512 * c + 128 * gam:512 * c + 128 * (gam + 1)] = \
                    scale * blk[128 * c:128 * (c + 1), :].T
    return arr


def _arrange_b(b_total):
    """-> (128, 512) f32: bb[32c+m, 128gam+j] = b[512*SRC[gam]+128c+j]."""
    bb = np.zeros((128, 512), np.float32)
    for c in range(4):
        row = np.empty(512, np.float32)
        for gam in range(4):
            scale = 2.0 if gam == 2 else 1.0
            row[128 * gam:128 * (gam + 1)] = \
                scale * b_total[512 * SRC[gam] + 128 * c:
                                512 * SRC[gam] + 128 * (c + 1)]
        bb[32 * c:32 * (c + 1), :] = row[None, :]
    return bb


def _arrange_pt(prot_half):
    """prot_half: (128, 512) protos for this direction's hidden half ->
    (4, 128, 128): pt[k][p, q] = prot[q, 128*(p//32) + 32k + (p%32)]."""
    hi = np.arange(128)
    cc, jp = hi // 32, hi % 32
    arr = np.empty((4, 128, 128), np.float32)
    for k in range(4):
        hin = 128 * cc + 32 * k + jp
        arr[k] = prot_half[:, hin].T
    return arr


def _make_selbig():
    """(128, 4*128): selbig[tt][p, 32c+m] = 1 if p == 32c + 8tt + m
    (and 8tt+m < 32) - routes granule-packed xg rows to G partitions."""
    sel = np.zeros((128, 4, 128), np.float32)
    for tt in range(4):
        for c in range(4):
            for m in range(32):
                p = 32 * c + 8 * tt + m
                if 8 * tt + m < 32:
                    sel[p, tt, 32 * c + m] = 1.0
    return sel.reshape(128, 512)


def _arrange_idx(ids_shard, n_gran):
    """ids_shard: (8, T) -> (32, n_gran) int32: [8*tt + b, g] = ids[b, 4g+tt]."""
    idx = np.zeros((32, n_gran), np.int32)
    for g in range(n_gran):
        for tt in range(4):
            for b in range(BS):
                idx[8 * tt + b, g] = ids_shard[b, 4 * g + tt]
    return idx


def build_program(n_gran=NG):
    nc = bacc.Bacc("TRN2", target_bir_lowering=False, debug=False)

    emb = nc.dram_tensor("emb", [V, E], F32, kind="ExternalInput").ap()
    idx_d = nc.dram_tensor("idx", [32, n_gran], I32, kind="ExternalInput").ap()
    wih_d = nc.dram_tensor("wih", [4, 128, 2048], BF16, kind="ExternalInput").ap()
    whh_d = nc.dram_tensor("whh", [4, 128, 2048], BF16, kind="ExternalInput").ap()
    bb_d = nc.dram_tensor("bb", [128, 512], F32, kind="ExternalInput").ap()
    pt_d = nc.dram_tensor("pt", [4, 128, 128], BF16, kind="ExternalInput").ap()
    sel_d = nc.dram_tensor("sel", [128, 512], BF16, kind="ExternalInput").ap()

    Tloc = 4 * n_gran
    xp_d = nc.dram_tensor("xp", [8, Tloc * OUTW], F32, kind="ExternalOutput").ap()

    with tile.TileContext(nc) as tc:
        _body(tc, n_gran, emb, idx_d, wih_d, whh_d, bb_d, pt_d, sel_d, xp_d)

    nc.compile()
    return nc


def _body(tc, n_gran, emb, idx_d, wih_d, whh_d, bb_d, pt_d, sel_d, xp_d):
    nc = tc.nc
    from contextlib import ExitStack
    ctx = ExitStack()
    const = ctx.enter_context(tc.tile_pool(name="const", bufs=1))
    state = ctx.enter_context(tc.tile_pool(name="state", bufs=1))
    work = ctx.enter_context(tc.tile_pool(name="work", bufs=2))
    psum_g = ctx.enter_context(tc.tile_pool(name="psg", bufs=2, space="PSUM"))
    psum_p = ctx.enter_context(tc.tile_pool(name="psp", bufs=2, space="PSUM"))
    psum_t = ctx.enter_context(tc.tile_pool(name="pst", bufs=1, space="PSUM"))
    psum_m = ctx.enter_context(tc.tile_pool(name="psm", bufs=1, space="PSUM"))

    # ---- resident tensors -------------------------------------------------
    wih = const.tile([128, 4 * 2048], BF16)
    whh = const.tile([128, 4 * 2048], BF16)
    bb = const.tile([128, 512], F32)
    pt = const.tile([128, 4 * 128], BF16)
    sel = const.tile([128, 512], BF16)
    idx = const.tile([32, n_gran], I32)
    ident = const.tile([128, 128], F32)

    for k in range(4):
        nc.sync.dma_start(wih[:, 2048 * k:2048 * (k + 1)], wih_d[k])
        nc.sync.dma_start(whh[:, 2048 * k:2048 * (k + 1)], whh_d[k])
        nc.sync.dma_start(pt[:, 128 * k:128 * (k + 1)], pt_d[k])
    nc.sync.dma_start(bb[:], bb_d[:])
    nc.sync.dma_start(sel[:], sel_d[:])
    nc.sync.dma_start(idx[:], idx_d[:])

    from concourse.masks import make_identity
    make_identity(nc, ident[:])

    # state
    c_st = state.tile([128, 128], F32)
    hT2 = state.tile([128, 2 * 128], BF16)          # ping-pong on t%2
    emb_ring = state.tile([32, 4 * 512], F32)       # slot = g%4
    embT = state.tile([128, 256], BF16)             # slot = g%2
    xg_ring = state.tile([128, 4 * 512], BF16)      # slot = g%4
    out_ring = state.tile([8, 16 * OUTW], F32)      # 16 steps per flush

    nc.gpsimd.memset(c_st[:], 0.0)
    nc.gpsimd.memset(hT2[:], 0.0)
    nc.gpsimd.memset(xg_ring[:], 0.0)
    nc.gpsimd.memset(emb_ring[:], 0.0)
    nc.gpsimd.memset(embT[:], 0.0)
    nc.gpsimd.memset(out_ring[:], 0.0)

    def gather(g):
        s = 512 * (g % 4)
        nc.gpsimd.indirect_dma_start(
            out=emb_ring[:, s:s + 512],
            out_offset=None,
            in_=emb[:],
            in_offset=bass.IndirectOffsetOnAxis(ap=idx[:, g:g + 1], axis=0),
        )

    def phase1(g):
        """embed transpose + xg GEMM (bias pre-injected) for granule g."""
        s, s2 = 512 * (g % 4), 128 * (g % 2)
        tp = psum_t.tile([128, 128], F32)
        for k in range(4):
            nc.tensor.matmul(
                tp[:, 32 * k:32 * k + 32],
                lhsT=emb_ring[:, s + 128 * k:s + 128 * (k + 1)],
                rhs=ident[:32, :32],
                is_transpose=True, start=(k == 0), stop=(k == 3))
        nc.scalar.copy(embT[:, s2:s2 + 128], tp[:])
        mm = psum_m.tile([128, 512], F32)
        # bias inject: one fp32 MM covering all 128 partitions
        nc.tensor.matmul(mm[:], lhsT=ident[:], rhs=bb[:],
                         start=True, stop=False)
        for k in range(4):
            for c in range(4):
                nc.tensor.matmul(
                    mm[32 * c:32 * c + 32, :],
                    lhsT=embT[:, s2 + 32 * k:s2 + 32 * k + 32],
                    rhs=wih[:, 2048 * k + 512 * c:2048 * k + 512 * (c + 1)],
                    start=False, stop=(k == 3),
                    tile_position=(0, 32 * c))
        nc.vector.tensor_copy(xg_ring[:, s:s + 512], mm[:])

    def step_mms(t):
        """xg inject + h@Whh for step t (reads hT2 buffer t%2)."""
        tt, slot = t % 4, 512 * ((t // 4) % 4)
        cur = hT2[:, 128 * (t % 2):128 * (t % 2) + 128]
        G = psum_g.tile([128, 512], F32, tag=f"g{t % 2}")
        nc.tensor.matmul(G[:], lhsT=sel[:, 128 * tt:128 * (tt + 1)],
                         rhs=xg_ring[:, slot:slot + 512],
                         start=True, stop=False)
        for k in range(4):
            for c in range(4):
                nc.tensor.matmul(
                    G[32 * c:32 * c + 32, :],
                    lhsT=cur[:, 32 * k:32 * k + 32],
                    rhs=whh[:, 2048 * k + 512 * c:2048 * k + 512 * (c + 1)],
                    start=False, stop=(k == 3),
                    tile_position=(0, 32 * c))
        return G

    def chain(t, G):
        """sigmoid -> cell update -> h -> hT2[(t+1)%2]."""
        nxt = hT2[:, 128 * ((t + 1) % 2):128 * ((t + 1) % 2) + 128]
        gh = work.tile([128, 512], F32, tag="gh")
        nc.scalar.activation(gh[:], G[:], mybir.ActivationFunctionType.Sigmoid)
        u = work.tile([128, 128], F32, tag="u")
        v = work.tile([128, 128], F32, tag="v")
        # u = (g' - 0.5) * i      (DVE)
        nc.vector.scalar_tensor_tensor(
            out=u[:], in0=gh[:, 256:384], scalar=0.5, in1=gh[:, 128:256],
            op0=mybir.AluOpType.subtract, op1=mybir.AluOpType.mult)
        # v = f * c               (GPSIMD, parallel with u)
        nc.gpsimd.tensor_tensor(out=v[:], in0=gh[:, 0:128], in1=c_st[:],
                                op=mybir.AluOpType.mult)
        # c = 2u + v              (DVE)
        nc.vector.scalar_tensor_tensor(
            out=c_st[:], in0=u[:], scalar=2.0, in1=v[:],
            op0=mybir.AluOpType.mult, op1=mybir.AluOpType.add)
        tc_t = work.tile([128, 128], F32, tag="tc")
        nc.scalar.activation(tc_t[:], c_st[:], mybir.ActivationFunctionType.Tanh)
        h_sb = work.tile([128, 128], BF16, tag="h")
        # h = o * tanh(c)         (DVE, bf16 out)
        nc.vector.tensor_tensor(out=h_sb[:], in0=gh[:, 384:512], in1=tc_t[:],
                                op=mybir.AluOpType.mult)
        # hT2 = blockwise 32x32 transpose of h (DVE stream transpose)
        nc.vector.transpose(nxt, h_sb[:])

    def proto_for_state(buf):
        """proto + Gram MMs against hT2 buffer `buf` (0/1)."""
        cur = hT2[:, 128 * buf:128 * buf + 128]
        pp = psum_p.tile([32, OUTW], F32)
        for k in range(4):
            nc.tensor.matmul(pp[:, 0:128],
                             lhsT=cur[:, 32 * k:32 * k + 32],
                             rhs=pt[:, 128 * k:128 * (k + 1)],
                             start=(k == 0), stop=False)
            nc.tensor.matmul(pp[:, 128:160],
                             lhsT=cur[:, 32 * k:32 * k + 32],
                             rhs=cur[:, 32 * k:32 * k + 32],
                             start=False, stop=(k == 3))
        return pp

    def emit_out(tprev, pp):
        nc.vector.tensor_copy(
            out_ring[:, OUTW * (tprev % 16):OUTW * (tprev % 16 + 1)],
            pp[0:8, :])
        if tprev % 16 == 15:
            blk = (tprev - 15) * OUTW
            nc.sync.dma_start(xp_d[0:8, blk:blk + 16 * OUTW], out_ring[:])

    # ---- main loop --------------------------------------------------------
    LOOKAHEAD = 2
    for g in range(min(LOOKAHEAD, n_gran)):
        gather(g)
        phase1(g)
    for g in range(n_gran):
        if g + LOOKAHEAD < n_gran:
            gather(g + LOOKAHEAD)
        for tt in range(4):
            t = 4 * g + tt
            G = step_mms(t)
            pp = proto_for_state(t % 2) if t > 0 else None
            chain(t, G)
            if pp is not None:
                emit_out(t - 1, pp)
        if g + LOOKAHEAD < n_gran:
            phase1(g + LOOKAHEAD)
    # final state (after step Tloc-1) lives in buffer Tloc%2
    pp = proto_for_state((4 * n_gran) % 2)
    emit_out(4 * n_gran - 1, pp)
    ctx.close()


def _prep_inputs(input_ids, embed_table, w_ih_f, w_hh_f, b_ih_f, b_hh_f,
                 w_ih_b, w_hh_b, b_ih_b, b_hh_b, prototypes, n_gran=NG):
    import ml_dtypes
    bf16 = ml_dtypes.bfloat16
    ids = np.asarray(input_ids).astype(np.int32)
    Tloc = 4 * n_gran
    emb = np.ascontiguousarray(np.asarray(embed_table, np.float32))
    prot = np.asarray(prototypes, np.float32)
    selb = _make_selbig().astype(bf16)
    per_dir = {}
    for d, (wi, wh, bi, bh) in enumerate([
            (w_ih_f, w_hh_f, b_ih_f, b_hh_f),
            (w_ih_b, w_hh_b, b_ih_b, b_hh_b)]):
        per_dir[d] = dict(
            wih=np.ascontiguousarray(
                _arrange_wih(np.asarray(wi, np.float32))).astype(bf16),
            whh=np.ascontiguousarray(
                _arrange_whh(np.asarray(wh, np.float32))).astype(bf16),
            bb=_arrange_b(np.asarray(bi, np.float32)
                          + np.asarray(bh, np.float32)),
            pt=np.ascontiguousarray(
                _arrange_pt(prot[:, 512 * d:512 * (d + 1)])).astype(bf16),
        )
    in_maps = []
    for core in range(8):
        d, shard = core // 4, core % 4
        ids_s = ids[8 * shard:8 * shard + 8, :Tloc]
        if d == 1:
            ids_s = ids_s[:, ::-1]
        in_maps.append(dict(
            emb=emb,
            idx=_arrange_idx(np.ascontiguousarray(ids_s), n_gran),
            wih=per_dir[d]["wih"], whh=per_dir[d]["whh"],
            bb=per_dir[d]["bb"], pt=per_dir[d]["pt"],
            sel=selb,
        ))
    return in_maps


def _combine(results, prototypes, n_gran=NG):
    Tloc = 4 * n_gran
    p2 = (np.asarray(prototypes, np.float32) ** 2).sum(-1)  # (128,)
    out = np.zeros((32, Tloc, 128), np.float32)
    bidx = np.arange(8)
    for core in range(8):
        d, shard = core // 4, core % 4
        blocks = results[core]["xp"].reshape(8, Tloc, OUTW)
        xp = blocks[:, :, 0:128]                       # (8, T, 128)
        x2 = blocks[bidx, :, 128 + bidx]               # (8, T)
        if d == 1:
            xp = xp[:, ::-1, :]
            x2 = x2[:, ::-1]
        sl = slice(8 * shard, 8 * shard + 8)
        out[sl] += 2.0 * xp - x2[:, :, None]
    out -= p2[None, None, :]
    return out


_NC_CACHE = {}


def kernel(input_ids, embed_table, w_ih_f, w_hh_f, b_ih_f, b_hh_f,
           w_ih_b, w_hh_b, b_ih_b, b_hh_b, prototypes):
    n_gran = NG
    if n_gran not in _NC_CACHE:
        _NC_CACHE[n_gran] = build_program(n_gran)
    nc = _NC_CACHE[n_gran]
    in_maps = _prep_inputs(input_ids, embed_table, w_ih_f, w_hh_f, b_ih_f,
                           b_hh_f, w_ih_b, w_hh_b, b_ih_b, b_hh_b, prototypes,
                           n_gran)
    res = run_bass_kernel_spmd(nc, in_maps, list(range(8)))
    return _combine(res.results, prototypes, n_gran)


if __name__ == "__main__":
    import time
    t0 = time.time()
    ng = int(sys.argv[1]) if len(sys.argv) > 1 else 8
    nc = build_program(ng)
    print(f"built n_gran={ng} in {time.time()-t0:.1f}s")


# revision 7
# speedup vs baseline: 1.3623x; 1.3623x over previous
"""BiLSTM + prototype-distance kernel for 8 trn2 NeuronCores.

Sharding: 8 cores = 4 batch-shards (8 rows each) x 2 directions.
Backward-direction cores receive time-reversed input ids and run the
identical SPMD program. Host combines per-core partial outputs:
    out = 2*(xp_f + xp_b) - x2_f - x2_b - ||protos||^2.

v3 design notes:
- Gate matmuls issue k-outer so the four 32-wide PE column strips
  stream concurrently.
- G is split across two PSUM banks: A=[f,i,g] (384 cols) and B=[o]
  (128 cols), each opened by one M=128 selector matmul (start=True
  covers every partition), so sigmoid(f,i,g) starts as soon as bank A
  closes while the o-gate matmuls finish behind it.
- Cell update: v=f*c, u=(g'-.5)*i, c=2u+v on DVE; tanh on ACT;
  h=o*tanh(c) on DVE (bf16); hT2 via DVE 32x32 stream transpose.
  Whh/protos are host-permuted so contraction chunk k reads
  hT2[:, 32k:32k+32] (partition p <-> hidden 128*(p//32)+32k+(p%32)).
- ||h||^2 via 4 extra N=32 Gram matmuls on the proto PSUM tile
  (host reads the diagonal).
- phase1 (x@Wih) is spread across the granule's 4 steps (one embed
  transpose + one k-round per step) so it never head-blocks the gate
  matmuls; the bank is opened by a bf16 zero-matmul and the bias is
  added exactly (fp32) by the PSUM->xg_ring cast STT.
- Embeds are cast to bf16 on GPSIMD so the embed transposes are bf16
  (fp32 matmuls run as two LOW/HIGH passes - 4x cost).
"""

import sys
import numpy as np

sys.path.insert(0, "/opt/trn_rl_repo")

import concourse.bass as bass  # noqa: E402
import concourse.tile as tile  # noqa: E402
import concourse.mybir as mybir  # noqa: E402
from concourse import bacc  # noqa: E402
from concourse.bass_utils import run_bass_kernel_spmd  # noqa: E402

F32 = mybir.dt.float32
BF16 = mybir.dt.bfloat16
I32 = mybir.dt.int32

V, E, HD, P = 50000, 512, 1024, 128
H2 = HD // 2          # 512 per-direction hidden
B, T = 32, 512
BS = 8                # batch rows per core
NG = T // 4           # granules (4 timesteps each)
# gate order in G columns: f, i, g | o  (pytorch rows are i,f,g,o)
SRC = [1, 0, 2, 3]
OUTW = 160            # 128 proto cols + 32 gram cols per step


def _whh_cols(w, k):
    """Columns of the k-th contraction chunk in the hT2 convention."""
    hi = np.arange(128)
    return 128 * (hi // 32) + 32 * k + (hi % 32)


def _arrange_whh(w):
    """w: (2048, 512) -> A: (4, 128, 4*384), B: (4, 128, 4*128)."""
    arrA = np.empty((4, 128, 4 * 384), np.float32)
    arrB = np.empty((4, 128, 4 * 128), np.float32)
    for k in range(4):
        hin = _whh_cols(w, k)
        for gam in range(4):
            scale = 2.0 if gam == 2 else 1.0
            blk = w[512 * SRC[gam]:512 * (SRC[gam] + 1), :][:, hin]  # (512,128)
            for c in range(4):
                sub = scale * blk[128 * c:128 * (c + 1), :].T        # (128,128)
                if gam < 3:
                    arrA[k, :, 384 * c + 128 * gam:384 * c + 128 * (gam + 1)] = sub
                else:
                    arrB[k, :, 128 * c:128 * (c + 1)] = sub
    return arrA, arrB


def _arrange_wih(w):
    """w: (2048, 512) -> (4, 128, 2048): arr[k][kk, 512c+128gam+j] =
    w[512*SRC[gam] + 128c + j, 128k + kk] (*2 on the tanh gate).
    Column order within a c-chunk: f, i, g, o."""
    arr = np.empty((4, 128, 2048), np.float32)
    for k in range(4):
        for gam in range(4):
            scale = 2.0 if gam == 2 else 1.0
            blk = w[512 * SRC[gam]:512 * (SRC[gam] + 1),
                    128 * k:128 * (k + 1)]      # (512, 128)
            for c in range(4):
                arr[k, :, 512 * c + 128 * gam:512 * c + 128 * (gam + 1)] = \
                    scale * blk[128 * c:128 * (c + 1), :].T
    return arr


def _arrange_b(b_total):
    """-> (128, 512) f32: bb[32c+m, 128gam+j] = b[512*SRC[gam]+128c+j]."""
    bb = np.zeros((128, 512), np.float32)
    for c in range(4):
        row = np.empty(512, np.float32)
        for gam in range(4):
            scale = 2.0 if gam == 2 else 1.0
            row[128 * gam:128 * (gam + 1)] = \
                scale * b_total[512 * SRC[gam] + 128 * c:
                                512 * SRC[gam] + 128 * (c + 1)]
        bb[32 * c:32 * (c + 1), :] = row[None, :]
    return bb


def _arrange_pt(prot_half):
    """prot_half: (128, 512) -> (4, 128, 128) in the hT2 convention."""
    hi = np.arange(128)
    cc, jp = hi // 32, hi % 32
    arr = np.empty((4, 128, 128), np.float32)
    for k in range(4):
        hin = 128 * cc + 32 * k + jp
        arr[k] = prot_half[:, hin].T
    return arr


def _make_selbig():
    """(128, 4*128): selbig[tt][p, 32c+m] = 1 if p == 32c + 8tt + m."""
    sel = np.zeros((128, 4, 128), np.float32)
    for tt in range(4):
        for c in range(4):
            for m in range(32):
                p = 32 * c + 8 * tt + m
                if 8 * tt + m < 32:
                    sel[p, tt, 32 * c + m] = 1.0
    return sel.reshape(128, 512)


def _arrange_idx(ids_shard, n_gran):
    idx = np.zeros((32, n_gran), np.int32)
    for g in range(n_gran):
        for tt in range(4):
            for b in range(BS):
                idx[8 * tt + b, g] = ids_shard[b, 4 * g + tt]
    return idx


def build_program(n_gran=NG):
    nc = bacc.Bacc("TRN2", target_bir_lowering=False, debug=False)

    emb = nc.dram_tensor("emb", [V, E], F32, kind="ExternalInput").ap()
    idx_d = nc.dram_tensor("idx", [32, n_gran], I32, kind="ExternalInput").ap()
    wih_d = nc.dram_tensor("wih", [4, 128, 2048], BF16, kind="ExternalInput").ap()
    whA_d = nc.dram_tensor("whA", [4, 128, 4 * 384], BF16, kind="ExternalInput").ap()
    whB_d = nc.dram_tensor("whB", [4, 128, 4 * 128], BF16, kind="ExternalInput").ap()
    bb_d = nc.dram_tensor("bb", [128, 512], F32, kind="ExternalInput").ap()
    pt_d = nc.dram_tensor("pt", [4, 128, 128], BF16, kind="ExternalInput").ap()
    sel_d = nc.dram_tensor("sel", [128, 512], BF16, kind="ExternalInput").ap()

    Tloc = 4 * n_gran
    xp_d = nc.dram_tensor("xp", [8, Tloc * OUTW], F32, kind="ExternalOutput").ap()

    with tile.TileContext(nc) as tc:
        _body(tc, n_gran, emb, idx_d, wih_d, whA_d, whB_d, bb_d, pt_d, sel_d,
              xp_d)

    nc.compile()
    return nc


def _body(tc, n_gran, emb, idx_d, wih_d, whA_d, whB_d, bb_d, pt_d, sel_d,
          xp_d):
    nc = tc.nc
    from contextlib import ExitStack
    ctx = ExitStack()
    const = ctx.enter_context(tc.tile_pool(name="const", bufs=1))
    state = ctx.enter_context(tc.tile_pool(name="state", bufs=1))
    work = ctx.enter_context(tc.tile_pool(name="work", bufs=2))
    psum_a = ctx.enter_context(tc.tile_pool(name="psa", bufs=2, space="PSUM"))
    psum_b = ctx.enter_context(tc.tile_pool(name="psb", bufs=2, space="PSUM"))
    psum_p = ctx.enter_context(tc.tile_pool(name="psp", bufs=2, space="PSUM"))
    psum_t = ctx.enter_context(tc.tile_pool(name="pst", bufs=1, space="PSUM"))
    psum_m = ctx.enter_context(tc.tile_pool(name="psm", bufs=1, space="PSUM"))

    # ---- resident tensors -------------------------------------------------
    wih = const.tile([128, 4 * 2048], BF16)
    whA = const.tile([128, 4 * 4 * 384], BF16)
    whB = const.tile([128, 4 * 4 * 128], BF16)
    bb = const.tile([128, 512], F32)
    pt = const.tile([128, 4 * 128], BF16)
    sel = const.tile([128, 512], BF16)
    idx = const.tile([32, n_gran], I32)
    identb = const.tile([32, 32], BF16)
    zeros = const.tile([128, 128], BF16)

    for k in range(4):
        nc.sync.dma_start(wih[:, 2048 * k:2048 * (k + 1)], wih_d[k])
        nc.sync.dma_start(whA[:, 1536 * k:1536 * (k + 1)], whA_d[k])
        nc.sync.dma_start(whB[:, 512 * k:512 * (k + 1)], whB_d[k])
        nc.sync.dma_start(pt[:, 128 * k:128 * (k + 1)], pt_d[k])
    nc.sync.dma_start(bb[:], bb_d[:])
    nc.sync.dma_start(sel[:], sel_d[:])
    nc.sync.dma_start(idx[:], idx_d[:])

    from concourse.masks import make_identity
    make_identity(nc, identb[:])
    nc.gpsimd.memset(zeros[:], 0.0)

    # state
    c_st = state.tile([128, 128], F32)
    hT2 = state.tile([128, 2 * 128], BF16)          # ping-pong on t%2
    emb_ring = state.tile([32, 4 * 512], F32)       # slot = g%4 (gather dst)
    embb_ring = state.tile([32, 4 * 512], BF16)     # bf16 cast of emb_ring
    embT = state.tile([128, 256], BF16)             # slot = g%2
    xgA_ring = state.tile([128, 4 * 384], BF16)     # slot = g%4
    xgB_ring = state.tile([128, 4 * 128], BF16)
    out_ring = state.tile([8, 16 * OUTW], F32)      # 16 steps per flush

    nc.gpsimd.memset(c_st[:], 0.0)
    nc.gpsimd.memset(hT2[:], 0.0)
    nc.gpsimd.memset(xgA_ring[:], 0.0)
    nc.gpsimd.memset(xgB_ring[:], 0.0)
    nc.gpsimd.memset(emb_ring[:], 0.0)
    nc.gpsimd.memset(embb_ring[:], 0.0)
    nc.gpsimd.memset(embT[:], 0.0)
    nc.gpsimd.memset(out_ring[:], 0.0)

    def gather(g):
        s = 512 * (g % 4)
        nc.gpsimd.indirect_dma_start(
            out=emb_ring[:, s:s + 512],
            out_offset=None,
            in_=emb[:],
            in_offset=bass.IndirectOffsetOnAxis(ap=idx[:, g:g + 1], axis=0),
        )
        nc.gpsimd.tensor_copy(embb_ring[:, s:s + 512], emb_ring[:, s:s + 512])

    # phase1 state carried across the granule's 4 chunks
    p1 = {}

    def phase1_chunk(g, k):
        """Chunk k of granule g's xg GEMM: embed-transpose k + MM round k."""
        s, s2 = 512 * (g % 4), 128 * (g % 2)
        if k == 0:
            p1[g] = psum_m.tile([128, 512], F32, name="p1mm")
            # open the bank: zero matmul covering all 128 partitions
            nc.tensor.matmul(p1[g][:], lhsT=zeros[:], rhs=wih[:, 0:512],
                             start=True, stop=False)
        tp = psum_t.tile([128, 32], BF16)
        nc.tensor.matmul(
            tp[:], lhsT=embb_ring[:, s + 128 * k:s + 128 * (k + 1)],
            rhs=identb[:], is_transpose=True, start=True, stop=True)
        nc.scalar.copy(embT[:, s2 + 32 * k:s2 + 32 * (k + 1)], tp[:])
        mm = p1[g]
        for c in range(4):
            nc.tensor.matmul(
                mm[32 * c:32 * c + 32, :],
                lhsT=embT[:, s2 + 32 * k:s2 + 32 * k + 32],
                rhs=wih[:, 2048 * k + 512 * c:2048 * k + 512 * (c + 1)],
                start=False, stop=(k == 3),
                tile_position=(0, 32 * c))
        if k == 3:
            # bias folded into the PSUM -> xg cast (exact fp32 bias)
            nc.vector.scalar_tensor_tensor(
                out=xgA_ring[:, 384 * (g % 4):384 * (g % 4) + 384],
                in0=mm[:, _A_COLS], scalar=1.0, in1=bb[:, _A_COLS],
                op0=mybir.AluOpType.mult, op1=mybir.AluOpType.add)
            nc.vector.scalar_tensor_tensor(
                out=xgB_ring[:, 128 * (g % 4):128 * (g % 4) + 128],
                in0=mm[:, _B_COLS], scalar=1.0, in1=bb[:, _B_COLS],
                op0=mybir.AluOpType.mult, op1=mybir.AluOpType.add)
            del p1[g]

    _A_COLS = slice(0, 384)
    _B_COLS = slice(384, 512)

    def step_mms(t):
        """xg inject + h@Whh for step t (reads hT2 buffer t%2)."""
        tt = t % 4
        sA, sB = 384 * ((t // 4) % 4), 128 * ((t // 4) % 4)
        cur = hT2[:, 128 * (t % 2):128 * (t % 2) + 128]
        GA = psum_a.tile([128, 384], F32, tag="ga")
        GB = psum_b.tile([128, 128], F32, tag="gb")
        nc.tensor.matmul(GA[:], lhsT=sel[:, 128 * tt:128 * (tt + 1)],
                         rhs=xgA_ring[:, sA:sA + 384],
                         start=True, stop=False)
        nc.tensor.matmul(GB[:], lhsT=sel[:, 128 * tt:128 * (tt + 1)],
                         rhs=xgB_ring[:, sB:sB + 128],
                         start=True, stop=False)
        for k in range(4):
            for c in range(4):
                nc.tensor.matmul(
                    GA[32 * c:32 * c + 32, :],
                    lhsT=cur[:, 32 * k:32 * k + 32],
                    rhs=whA[:, 1536 * k + 384 * c:1536 * k + 384 * (c + 1)],
                    start=False, stop=(k == 3),
                    tile_position=(0, 32 * c))
        for k in range(4):
            for c in range(4):
                nc.tensor.matmul(
                    GB[32 * c:32 * c + 32, :],
                    lhsT=cur[:, 32 * k:32 * k + 32],
                    rhs=whB[:, 512 * k + 128 * c:512 * k + 128 * (c + 1)],
                    start=False, stop=(k == 3),
                    tile_position=(0, 32 * c))
        return GA, GB

    def chain(t, GA, GB):
        nxt = hT2[:, 128 * ((t + 1) % 2):128 * ((t + 1) % 2) + 128]
        gh = work.tile([128, 384], F32, tag="gh")
        gho = work.tile([128, 128], F32, tag="gho")
        nc.scalar.activation(gh[:], GA[:], mybir.ActivationFunctionType.Sigmoid)
        nc.scalar.activation(gho[:], GB[:], mybir.ActivationFunctionType.Sigmoid)
        u = work.tile([128, 128], F32, tag="u")
        v = work.tile([128, 128], F32, tag="v")
        # v = f * c  (DVE)
        nc.vector.tensor_tensor(out=v[:], in0=gh[:, 0:128], in1=c_st[:],
                                op=mybir.AluOpType.mult)
        # u = (g' - 0.5) * i
        nc.vector.scalar_tensor_tensor(
            out=u[:], in0=gh[:, 256:384], scalar=0.5, in1=gh[:, 128:256],
            op0=mybir.AluOpType.subtract, op1=mybir.AluOpType.mult)
        # c = 2u + v
        nc.vector.scalar_tensor_tensor(
            out=c_st[:], in0=u[:], scalar=2.0, in1=v[:],
            op0=mybir.AluOpType.mult, op1=mybir.AluOpType.add)
        tc_t = work.tile([128, 128], F32, tag="tc")
        nc.scalar.activation(tc_t[:], c_st[:], mybir.ActivationFunctionType.Tanh)
        h_sb = work.tile([128, 128], BF16, tag="h")
        nc.vector.tensor_tensor(out=h_sb[:], in0=gho[:], in1=tc_t[:],
                                op=mybir.AluOpType.mult)
        nc.vector.transpose(nxt, h_sb[:])

    def proto_for_state(buf):
        cur = hT2[:, 128 * buf:128 * buf + 128]
        pp = psum_p.tile([32, OUTW], F32)
        for k in range(4):
            nc.tensor.matmul(pp[:, 0:128],
                             lhsT=cur[:, 32 * k:32 * k + 32],
                             rhs=pt[:, 128 * k:128 * (k + 1)],
                             start=(k == 0), stop=False)
            nc.tensor.matmul(pp[:, 128:160],
                             lhsT=cur[:, 32 * k:32 * k + 32],
                             rhs=cur[:, 32 * k:32 * k + 32],
                             start=False, stop=(k == 3))
        return pp

    def emit_out(tprev, pp):
        nc.vector.tensor_copy(
            out_ring[:, OUTW * (tprev % 16):OUTW * (tprev % 16 + 1)],
            pp[0:8, :])
        if tprev % 16 == 15:
            blk = (tprev - 15) * OUTW
            nc.sync.dma_start(xp_d[0:8, blk:blk + 16 * OUTW], out_ring[:])

    # ---- main loop --------------------------------------------------------
    LOOKAHEAD = 2
    for g in range(min(LOOKAHEAD, n_gran)):
        gather(g)
        for k in range(4):
            phase1_chunk(g, k)
    for g in range(n_gran):
        if g + LOOKAHEAD < n_gran:
            gather(g + LOOKAHEAD)
        for tt in range(4):
            t = 4 * g + tt
            GA, GB = step_mms(t)
            pp = proto_for_state(t % 2) if t > 0 else None
            if g + LOOKAHEAD < n_gran:
                phase1_chunk(g + LOOKAHEAD, tt)
            chain(t, GA, GB)
            if pp is not None:
                emit_out(t - 1, pp)
    pp = proto_for_state((4 * n_gran) % 2)
    emit_out(4 * n_gran - 1, pp)
    ctx.close()


def _prep_inputs(input_ids, embed_table, w_ih_f, w_hh_f, b_ih_f, b_hh_f,
                 w_ih_b, w_hh_b, b_ih_b, b_hh_b, prototypes, n_gran=NG):
    import ml_dtypes
    bf16 = ml_dtypes.bfloat16
    ids = np.asarray(input_ids).astype(np.int32)
    Tloc = 4 * n_gran
    emb = np.ascontiguousarray(np.asarray(embed_table, np.float32))
    prot = np.asarray(prototypes, np.float32)
    selb = _make_selbig().astype(bf16)
    per_dir = {}
    for d, (wi, wh, bi, bh) in enumerate([
            (w_ih_f, w_hh_f, b_ih_f, b_hh_f),
            (w_ih_b, w_hh_b, b_ih_b, b_hh_b)]):
        whA, whB = _arrange_whh(np.asarray(wh, np.float32))
        per_dir[d] = dict(
            wih=np.ascontiguousarray(
                _arrange_wih(np.asarray(wi, np.float32))).astype(bf16),
            whA=np.ascontiguousarray(whA).astype(bf16),
            whB=np.ascontiguousarray(whB).astype(bf16),
            bb=_arrange_b(np.asarray(bi, np.float32)
                          + np.asarray(bh, np.float32)),
            pt=np.ascontiguousarray(
                _arrange_pt(prot[:, 512 * d:512 * (d + 1)])).astype(bf16),
        )
    in_maps = []
    for core in range(8):
        d, shard = core // 4, core % 4
        ids_s = ids[8 * shard:8 * shard + 8, :Tloc]
        if d == 1:
            ids_s = ids_s[:, ::-1]
        in_maps.append(dict(
            emb=emb,
            idx=_arrange_idx(np.ascontiguousarray(ids_s), n_gran),
            wih=per_dir[d]["wih"], whA=per_dir[d]["whA"],
            whB=per_dir[d]["whB"],
            bb=per_dir[d]["bb"], pt=per_dir[d]["pt"],
            sel=selb,
        ))
    return in_maps


def _combine(results, prototypes, n_gran=NG):
    Tloc = 4 * n_gran
    p2 = (np.asarray(prototypes, np.float32) ** 2).sum(-1)  # (128,)
    out = np.zeros((32, Tloc, 128), np.float32)
    bidx = np.arange(8)
    for core in range(8):
        d, shard = core // 4, core % 4
        blocks = results[core]["xp"].reshape(8, Tloc, OUTW)
        xp = blocks[:, :, 0:128]                       # (8, T, 128)
        x2 = blocks[bidx, :, 128 + bidx]               # (8, T)
        if d == 1:
            xp = xp[:, ::-1, :]
            x2 = x2[:, ::-1]
        sl = slice(8 * shard, 8 * shard + 8)
        out[sl] += 2.0 * xp - x2[:, :, None]
    out -= p2[None, None, :]
    return out


_NC_CACHE = {}


def kernel(input_ids, embed_table, w_ih_f, w_hh_f, b_ih_f, b_hh_f,
           w_ih_b, w_hh_b, b_ih_b, b_hh_b, prototypes):
    n_gran = NG
    if n_gran not in _NC_CACHE:
        _NC_CACHE[n_gran] = build_program(n_gran)
    nc = _NC_CACHE[n_gran]
    in_maps = _prep_inputs(input_ids, embed_table, w_ih_f, w_hh_f, b_ih_f,
                           b_hh_f, w_ih_b, w_hh_b, b_ih_b, b_hh_b, prototypes,
                           n_gran)
    res = run_bass_kernel_spmd(nc, in_maps, list(range(8)))
    return _combine(res.results, prototypes, n_gran)


if __name__ == "__main__":
    import time
    t0 = time.time()
    ng = int(sys.argv[1]) if len(sys.argv) > 1 else 8
    nc = build_program(ng)
    print(f"built n_gran={ng} in {time.time()-t0:.1f}s")
